# revision 25
# baseline (speedup 1.0000x reference)
"""HadamardNorm kernel for Trainium2 (8 NeuronCores, pure data parallel).

Computes y = LeakyReLU_{0.1}( FWHT_4096(x) / sqrt(4096) ) row-wise on
x of shape (4, 4096, 4096) fp32.

Math: FWHT_4096 = H32 (x) H2 (x) H2 (x) H32 over index bits
(i:5 | j1:1 | j0:1 | jl:5).  Per 16 rows (s in 4, g in 4), tile
[128 part, 512 free]:

  IN   [(s,i) part, (g, j=128) free] bf16  <- DMA, 256B contiguous runs
  T0   [(s,jl), (g, j1, j0, i)]            <- DVE 32x32 stream transpose
  MMA  [(s,bl), (g, j1, b0, i)]            <- PE: 4 accumulating bf16
                                              matmuls (N=256) fold j0
                                              with +-W, contracting jl
  T1   [(s,i), (g, j1, b0, bl)]            <- DVE stream transpose (PSUM)
  FOLD [(s,i), (g, b1, b0, bl)] bf16       <- GpSimd/DVE: H2 on j1
  MMB  [(s,a), (g, b=128)]                 <- PE: W contracts i (N=512)
  OUT  Lrelu(scale=1/64, alpha=0.1) bf16   <- ACT (PSUM drain + cast)
  y    <- DMA store bf16, 256B runs

All matmuls are bf16 (H entries +-1 are exact; PSUM accumulates f32).
x is cast to bf16 host-side; y returns bf16 upcast to f32.  End-to-end
error ~3e-3 of max |y| against the 2e-2 tolerance.
"""

import numpy as np
import ml_dtypes

import concourse.bass as bass
import concourse.mybir as mybir
import concourse.tile as tile
from concourse import bacc
from concourse.bass_utils import run_bass_kernel_spmd

N_CORES = 8
D = 4096
ROWS_TOTAL = 4 * 4096              # 16384 rows of 4096
ROWS_PER_CORE = ROWS_TOTAL // N_CORES  # 2048

F32 = mybir.dt.float32
BF16 = mybir.dt.bfloat16

B = 2                              # iters per DMA superblock (32 rows)
N_SGB = ROWS_PER_CORE // (16 * B)  # 64 superblocks per core
FOLD_DVE_MOD = 16                  # every Nth iter folds on DVE (0=never)


def _hadamard(n: int) -> np.ndarray:
    h = np.array([[1.0]], dtype=np.float32)
    while h.shape[0] < n:
        h = np.block([[h, h], [h, -h]])
    return h.astype(np.float32)


def _build_nc():
    H32 = _hadamard(32)
    WP = np.kron(np.eye(4, dtype=np.float32), H32)    # [128,128]
    WM = -WP

    nc = bacc.Bacc("TRN2", target_bir_lowering=False, debug=False,
                   num_devices=N_CORES)

    # row = sgb*64 + u*16 + g*4 + s ; col = i*128 + j
    x = nc.dram_tensor("x", [N_SGB, B, 4, 4, 32, 128], BF16,
                       kind="ExternalInput")
    y = nc.dram_tensor("y", [N_SGB, B, 4, 4, 32, 128], BF16,
                       kind="ExternalOutput")

    wp_d = nc.inline_tensor(WP.astype(ml_dtypes.bfloat16), "wpc")
    wm_d = nc.inline_tensor(WM.astype(ml_dtypes.bfloat16), "wmc")

    with tile.TileContext(nc) as tc:
        with (
            tc.tile_pool(name="wpool", bufs=1) as wpool,
            tc.tile_pool(name="inp", bufs=4) as inp,
            tc.tile_pool(name="t0p", bufs=6) as t0p,
            tc.tile_pool(name="psap", bufs=5, space="PSUM") as psap,
            tc.tile_pool(name="t2p", bufs=4) as t2p,
            tc.tile_pool(name="vp", bufs=4) as vp,
            tc.tile_pool(name="psbp", bufs=3, space="PSUM") as psbp,
            tc.tile_pool(name="outp", bufs=4) as outp,
        ):
            wp = wpool.tile([128, 128], BF16, tag="wp")
            wm = wpool.tile([128, 128], BF16, tag="wm")
            nc.sync.dma_start(wp[:], wp_d[:])
            nc.sync.dma_start(wm[:], wm_d[:])
            wpr = wp[:]
            wmr = wm[:]

            def front_half(sgb):
                """Load + T0 + MMA for superblock sgb; returns psa list."""
                tin = inp.tile([128, 512 * B], BF16, tag="tin")
                src = x[sgb].rearrange("u g s i j -> (s i) (u g) j")
                nc.sync.dma_start(
                    tin[:].rearrange(
                        "p (ug j) -> p ug j", ug=4 * B, j=128), src)
                t0s = []
                for u in range(B):
                    # T0: [(s,i),(g,j1,j0,jl)] -> [(s,jl), j0-major free]
                    # t0 physical free layout (j0, g, j1, i) so MMA rhs
                    # slices are contiguous.
                    t0 = t0p.tile([128, 512], BF16, tag="t0")
                    nc.vector.transpose(
                        t0[:].rearrange("p (j0 g j1 i) -> p g j1 j0 i",
                                        j0=2, g=4, j1=2, i=32),
                        tin[:, u * 512:(u + 1) * 512].rearrange(
                            "p (g j1 j0 jl) -> p g j1 j0 jl",
                            g=4, j1=2, j0=2, jl=32))
                    t0s.append(t0)

                psas = []
                for u in range(B):
                    t0v = t0s[u][:].rearrange("p (j0 c) -> p j0 c",
                                              j0=2, c=256)
                    # psa physical free layout (b0, g, j1, i): contiguous
                    # matmul output slices
                    psa = psap.tile([128, 512], F32, tag="psa")
                    psav = psa[:].rearrange("p (b0 c) -> p b0 c",
                                            b0=2, c=256)
                    # b0=0: +j0=0 +j0=1 ; b0=1: +j0=0 -j0=1
                    nc.tensor.matmul(psav[:, 0], wpr, t0v[:, 0],
                                     start=True, stop=False)
                    nc.tensor.matmul(psav[:, 0], wpr, t0v[:, 1],
                                     start=False, stop=True)
                    nc.tensor.matmul(psav[:, 1], wpr, t0v[:, 0],
                                     start=True, stop=False)
                    nc.tensor.matmul(psav[:, 1], wmr, t0v[:, 1],
                                     start=False, stop=True)
                    psas.append(psa)
                return psas

            def back_half(sgb, psas):
                """T1 + fold + MMB + Prelu + store for superblock sgb."""
                t2s = []
                for u in range(B):
                    # T1: [(s,bl),(b0,g,j1,i)] -> [(s,i), t2 (g,j1,b0,bl)]
                    t2 = t2p.tile([128, 512], F32, tag="t2")
                    nc.vector.transpose(
                        t2[:].rearrange("p (g j1 b0 bl) -> p b0 g j1 bl",
                                        g=4, j1=2, b0=2, bl=32),
                        psas[u][:].rearrange("p (b0 g j1 i) -> p b0 g j1 i",
                                             b0=2, g=4, j1=2, i=32))
                    t2s.append(t2)

                vs = []
                for u in range(B):
                    it = sgb * B + u
                    # H2 fold on j1 -> bf16 for MMB's ifmap
                    v = vp.tile([128, 512], BF16, tag="v")
                    t2v = t2s[u][:].rearrange("p (g j1 c) -> p j1 g c",
                                              g=4, j1=2, c=64)
                    vv = v[:].rearrange("p (g b1 c) -> p b1 g c",
                                        g=4, b1=2, c=64)
                    eng = (nc.vector if (FOLD_DVE_MOD and
                                         it % FOLD_DVE_MOD == 0)
                           else nc.gpsimd)
                    eng.tensor_add(vv[:, 0], t2v[:, 0], t2v[:, 1])
                    eng.tensor_sub(vv[:, 1], t2v[:, 0], t2v[:, 1])
                    vs.append(v)

                for u in range(B):
                    # MMB: contract i -> [(s,a),(g,b)]
                    psb = psbp.tile([128, 512], F32, tag="psb")
                    nc.tensor.matmul(psb[:], wpr, vs[u][:],
                                     start=True, stop=True)
                    tout = outp.tile([128, 512], BF16, tag="tout")
                    nc.scalar.activation(
                        tout[:],
                        psb[:],
                        mybir.ActivationFunctionType.Prelu,
                        bias=0.0, scale=1.0 / 64.0, alpha=0.1)
                    dst = y[sgb, u].rearrange("g s a b -> (s a) g b")
                    nc.sync.dma_start(
                        dst, tout[:].rearrange("p (g b) -> p g b",
                                               g=4, b=128))

            # software pipeline, lag 1: back half of sgb-1 is emitted
            # after the front half of sgb, so every op is data-ready
            # when it reaches the head of its engine's FIFO.
            pending = None
            for sgb in range(N_SGB + 1):
                psas = front_half(sgb) if sgb < N_SGB else None
                if pending is not None:
                    back_half(pending[0], pending[1])
                pending = (sgb, psas) if sgb < N_SGB else None
    nc.finalize()
    return nc


_NC_CACHE = {}


def _get_nc():
    if "nc" not in _NC_CACHE:
        _NC_CACHE["nc"] = _build_nc()
    return _NC_CACHE["nc"]


def run(x: np.ndarray, trace: bool = False):
    """Returns (y, BassKernelResults)."""
    x = np.ascontiguousarray(x, dtype=np.float32)
    flat = x.reshape(-1, D).astype(ml_dtypes.bfloat16)
    dev_shape = (N_SGB, B, 4, 4, 32, 128)
    shards = [
        np.ascontiguousarray(
            flat[c * ROWS_PER_CORE:(c + 1) * ROWS_PER_CORE]).reshape(dev_shape)
        for c in range(N_CORES)
    ]
    nc = _get_nc()
    res = run_bass_kernel_spmd(
        nc, [{"x": s} for s in shards], core_ids=list(range(N_CORES)),
        trace=trace)
    out = np.concatenate(
        [np.asarray(r["y"]).astype(np.float32).reshape(ROWS_PER_CORE, D)
         for r in res.results], axis=0)
    return out.reshape(x.shape), res


def kernel(x: np.ndarray) -> np.ndarray:
    out, _ = run(x, trace=False)
    return out
